# revision 12
# baseline (speedup 1.0000x reference)
"""STSPBlock Trainium2 kernel, v2.

Per core (batch-sharded B=16 -> 8 cores x B=2), partitions p = b*64+c.

Key points vs v1 (685us -> ~205us est., hw rel err 3.4e-3):
  - conv0 + node convs run in bf16 (1 cyc/row vs 4 for fp32).
  - conv0 gets near-fp32 precision for free by packing a split-bf16
    product into spare K rows: K=109 = ones + Whi*xhi + Wlo*xhi +
    Whi*xlo (im2col built host-side, one 8KB-run DMA per block).
    Node-conv bf16 weight rounding was measured to flip zero spikes
    (integer-valued rhs -> wide thresholds margins), so single-pass.
  - LIF membranes stay exact fp32: u = -(1-c)/2*state + conv via one
    DVE scalar_tensor_tensor reading PSUM (replaces identity matmuls,
    whose reduced-precision would flip spikes; fp32 matmul too slow).
    State is kept as (sign-1)*u so the GPSIMD reset needs only
    mult/subtract tensor_tensor ops (Pool HW has no compare/scalar).
  - spikes computed on ACT as sign(u-1) in {-1,+1}; consumers are
    linear so affine corrections fold into host constants. accum_out
    of the sign op yields the per-node spike counts for free.
  - y accumulates in PSUM fp32 via scaled-identity bf16 matmuls; the
    constant row is built from the *rounded* per-node scales (split
    hi+lo) so no-spike pixels cancel exactly.
  - alpha = f(graph attention) needs <1e-3 accuracy: rsqrt computed
    by reciprocal-seeded Newton (2 iters, mult-only; DVE pow and the
    bit-trick are not HW-legal, Ln/Exp thrash activation tables).
  - graph chain shortened (gat_w @ gat_a folded host-side), single
    activation table load, bias matmuls skipped when zero.
"""

import numpy as np
import ml_dtypes

import concourse.bass as bass
import concourse.bacc as bacc
import concourse.mybir as mybir
from concourse.tile import TileContext
from concourse.bass_utils import run_bass_kernel_spmd

FP = mybir.dt.float32
FR = mybir.dt.float32r
BF = mybir.dt.bfloat16
Alu = mybir.AluOpType
Act = mybir.ActivationFunctionType

T, BFULL, CIN, H, W = 8, 16, 2, 64, 64
CO, NN, HEADS = 64, 4, 4
HP, WP = 32, 32
BC = 2
NCORES = 8
EPS = 1e-5
DECAY = 0.6
HD = CO // HEADS
XPT = 72 * 4096         # host-im2col rows (hi 36 + lo 36) per step


# ----------------------------------------------------------------- host consts
def _host_consts(conv0_w, bn0_g, bn0_b, bn0_m, bn0_v, lif0_w,
                 convs_w, bns_g, bns_b, bns_m, bns_v, lifs_w,
                 ft_w, ft_b, gat_w, gat_a, out_weights):
    f32 = np.float32
    bf = ml_dtypes.bfloat16
    sig = lambda z: 1.0 / (1.0 + np.exp(-np.asarray(z, np.float64)))
    c0 = f32(sig(lif0_w))
    cn = sig(lifs_w).astype(f32)          # [3]
    ws = sig(out_weights).astype(f32)     # [4]

    s0c = (bn0_g / np.sqrt(bn0_v + EPS)).astype(f32)
    bias0 = ((bn0_b - bn0_m * s0c) * c0).astype(f32)
    W0f = (conv0_w * s0c[:, None, None, None] * c0).astype(f32)  # [64,2,3,3]

    # conv0 folded lhsT [109, 128], split-bf16 via spare K rows:
    #   rows 1-36:  Whi (pairs with xhi), rows 37-72: Wlo (pairs with xhi),
    #   rows 73-108: Whi (pairs with xlo); row 0 = bias (ones row).
    # Dropped Wlo*xlo term is O(2^-16) relative.
    W0hi = W0f.astype(bf).astype(f32)
    W0lo = (W0f - W0hi).astype(bf).astype(f32)
    w0bd = np.zeros((109, 128), f32)
    for dy in range(3):
        for dx in range(3):
            for b in range(2):
                for ci in range(2):
                    p = 1 + dy * 12 + dx * 4 + b * 2 + ci
                    w0bd[p, b * 64:(b + 1) * 64] = W0hi[:, ci, dy, dx]
                    w0bd[36 + p, b * 64:(b + 1) * 64] = W0lo[:, ci, dy, dx]
                    w0bd[72 + p, b * 64:(b + 1) * 64] = W0hi[:, ci, dy, dx]
    w0bd[0, 0:64] = bias0
    w0bd[0, 64:128] = bias0

    sncol = (bns_g / np.sqrt(bns_v + EPS)).astype(f32)            # [3,64]
    biasn_raw = (bns_b - bns_m * sncol).astype(f32)               # [3,64]
    # 0.125 = avgpool fold (out0p holds 2x the SUM of 4 spikes)
    Wf = (convs_w * sncol[:, :, None, None, None] * 0.125).astype(f32)

    wnod = np.zeros((3, 9, 128, 128), f32)
    for n in range(3):
        for dy in range(3):
            for dx in range(3):
                k = dy * 3 + dx
                blk = Wf[n, :, :, dy, dx].T    # [ci, co]
                wnod[n, k, 0:64, 0:64] = blk
                wnod[n, k, 64:128, 64:128] = blk

    biasn = np.concatenate([np.tile(cn[n] * biasn_raw[n], 2)
                            for n in range(3)]).reshape(1, 384).astype(f32)

    def bd(m):  # block-diag [128,128] of m.T twice ([co,ci] -> lhsT)
        z = np.zeros((128, 128), f32)
        z[0:64, 0:64] = m.T
        z[64:128, 64:128] = m.T
        return z

    # f0: pre-relu = ft_w @ (f0sum/4096) + ft_b          (f0sum: true sums)
    # fn: pre-relu = ft_w @ (snsum/2048) + ft_b + 0.5*ft_w.sum(1)
    #     (snsum = sum over pixels of sign values in {-1,+1})
    ftmm = np.stack([bd(ft_w * (0.4 / 8192.0)), bd(ft_w * (0.4 / 2048.0))])
    ftb0 = (0.4 * np.tile(ft_b, 2)).reshape(128, 1).astype(f32)
    ftb1 = (0.4 * np.tile(ft_b + 0.5 * ft_w.sum(axis=1), 2)).reshape(
        128, 1).astype(f32)

    gwbd = bd(gat_w).astype(f32)
    ga1 = np.zeros((128, 8), f32)
    ga2 = np.zeros((128, 8), f32)
    for b in range(2):
        for h in range(HEADS):
            for d in range(HD):
                ga1[b * 64 + h * 16 + d, b * 4 + h] = gat_a[h, d]
                ga2[b * 64 + h * 16 + d, b * 4 + h] = gat_a[h, HD + d]
    M1 = (gwbd @ ga1).astype(f32)          # [128, 8] folded gat_w @ a_l
    M2 = (gwbd @ ga2).astype(f32)
    # one MM: out partitions 0-7 = e1, 32-39 = e2 (PSUM reads need
    # 32-aligned base partitions)
    M12 = np.zeros((128, 40), f32)
    M12[:, 0:8] = M1
    M12[:, 32:40] = M2

    ghbd = np.zeros((8, 2), f32)
    for b in range(2):
        ghbd[b * 4:(b + 1) * 4, b] = 12.5

    gbc = np.zeros((2, 128), f32)
    gbc[0, 0:64] = 1.0
    gbc[1, 64:128] = 1.0

    cnrow = np.tile(cn[None, :], (2, 1)).astype(f32)              # [2,3]

    def cols(stk):  # [k,128,128] -> [128, k*128]
        return np.ascontiguousarray(
            np.transpose(stk, (1, 0, 2)).reshape(128, -1))

    # y identity scales are bf16 in the MM; the constant row must cancel
    # the ROUNDED per-node scales exactly for no-spike pixels, so build it
    # from the bf16-rounded values and store it split hi+lo (K=2 rows).
    wnb = [f32(bf(f32(ws[n] * 0.5))) for n in (1, 2, 3)]
    iy = np.stack([float(ws[0]) * 0.125 * np.eye(128),
                   wnb[0] * np.eye(128),
                   wnb[1] * np.eye(128),
                   wnb[2] * np.eye(128)]).astype(f32)
    yC = f32(wnb[0]) + f32(wnb[1]) + f32(wnb[2])
    yChi = f32(bf(yC))
    yClo = f32(bf(yC - yChi))
    ycrow = np.zeros((2, 128), f32)
    ycrow[0, :] = yChi
    ycrow[1, :] = yClo

    consts = dict(
        w0bd=w0bd.astype(bf),
        wnod=cols(wnod.reshape(27, 128, 128)).astype(bf),
        biasn=biasn,
        ftmm=cols(ftmm), ftb0=ftb0, ftb1=ftb1,
        m12=M12, ghbd=ghbd, gbc=gbc, cnrow=cnrow,
        iy=cols(iy).astype(bf), ycrow=ycrow.astype(bf))

    # y scales: y = ws0*(out0sum/4) + sum_n wsn*(sn_sign+1)/2
    # plus LIF leak factors for the on-chip u-ops
    yw = (float(ws[0]) * 0.125,
          float(ws[1]) * 0.5, float(ws[2]) * 0.5, float(ws[3]) * 0.5,
          float(0.5 * (ws[1] + ws[2] + ws[3])),
          float(1.0 - c0), float(1.0 - cn[0]), float(1.0 - cn[1]),
          float(1.0 - cn[2]))
    flags = (bool(np.any(np.abs(biasn) > 0)),)
    return consts, yw, flags


CONST_SPECS = dict(w0bd=((109, 128), BF),
                   wnod=((128, 27 * 128), BF),
                   biasn=((1, 384), FP), ftmm=((128, 2 * 128), FP),
                   ftb0=((128, 1), FP), ftb1=((128, 1), FP),
                   m12=((128, 40), FP),
                   ghbd=((8, 2), FP), gbc=((2, 128), FP),
                   cnrow=((2, 3), FP), iy=((128, 4 * 128), BF),
                   ycrow=((2, 128), BF))


# ------------------------------------------------------------------ the module
DBG = False


def build_nc(yw, biasn_nz):
    nc = bacc.Bacc(None, target_bir_lowering=False)
    xpad = nc.declare_dram_parameter("xpad", [T * XPT], BF, isOutput=False)
    cst = {k: nc.declare_dram_parameter(k, list(shp), dt, isOutput=False)
           for k, (shp, dt) in CONST_SPECS.items()}
    y = nc.declare_dram_parameter("y", [T, BC, CO, HP, WP], FP, isOutput=True)
    dbg = {}
    if DBG:
        for nm, shp, dt in [("d_s0", [128, 4096], BF), ("d_o0", [128, 1156], BF),
                        ("d_f0sum", [128, 1], FP), ("d_f0t", [128, 1], FP),
                        ("d_Tt", [128, 4], FP), ("d_es", [8, 16], FP),
                        ("d_S", [2, 16], FP), ("d_aap", [128, 3], FP),
                        ("d_q", [2, 4], FP), ("d_s1", [128, 3072], BF),
                        ("d_snsum", [128, 6], FP), ("d_u1", [128, 4096], FP)]:
            dbg[nm] = nc.declare_dram_parameter(nm, shp, dt, isOutput=True)

    w0s4, w1, w2, w3, yC, l0, ln1, ln2, ln3 = yw
    wns = (w1, w2, w3)
    lns = (ln1, ln2, ln3)

    with TileContext(nc) as tc:
        with (
            tc.tile_pool(name="consts", bufs=1) as cpool,
            tc.tile_pool(name="state", bufs=1) as spool,
            tc.tile_pool(name="im", bufs=2) as impool,
            tc.tile_pool(name="big", bufs=2) as bpool,
            tc.tile_pool(name="sw", bufs=2) as swpool,
            tc.tile_pool(name="tiny", bufs=4) as tpool,
            tc.tile_pool(name="pconv", bufs=5, space="PSUM") as ps_conv,
            tc.tile_pool(name="pnode", bufs=2, space="PSUM") as ps_node,
            tc.tile_pool(name="ptiny", bufs=1, space="PSUM") as ps_tiny,
        ):
            # ---- consts to SBUF (w0bd first on sync; rest on ACT queue
            # so the first conv0 + im DMAs aren't stuck behind them)
            csb = {}
            for k, (shp, dt) in CONST_SPECS.items():
                t_ = cpool.tile(list(shp), dt, tag=k)
                (nc.sync if k == "w0bd" else nc.scalar).dma_start(
                    t_[:], cst[k][:])
                csb[k] = t_

            ones = None
            if biasn_nz:
                ones = cpool.tile([1, 512], FP, tag="ones")
                nc.vector.memset(ones[:], 1.0)

            # activation biases must be APs: [-1.0 (sign), 1e-6 (ln), 0.0]
            actc = cpool.tile([128, 3], FP, tag="actc")
            nc.vector.memset(actc[:, 0:1], -1.0)
            nc.vector.memset(actc[:, 1:2], 1e-6)
            nc.vector.memset(actc[:, 2:3], 0.0)

            # ---- states (in-place: u-op reads v before reset rewrites it)
            v0 = spool.tile([128, 4096], FP, tag="v0")
            vn = spool.tile([128, 3072], FP, tag="vn")
            u0t = spool.tile([128, 4096], FP, tag="u0t")
            unt = spool.tile([128, 3072], FP, tag="unt")
            Tt = spool.tile([128, 4], FP, tag="Tt")
            nc.vector.memset(Tt[:], 0.0)
            nc.vector.memset(vn[:], 0.0)

            # out0p (true spike-sum domain, zero border = zero pad)
            o0A = spool.tile([128, 34 * 34], BF, tag="o0A")
            o0B = spool.tile([128, 34 * 34], BF, tag="o0B")
            nc.vector.memset(o0A[:], 0.0)
            nc.vector.memset(o0B[:], 0.0)
            yones = cpool.tile([2, 512], BF, tag="yones")
            nc.vector.memset(yones[:], 1.0)

            imA = impool.tile([109, 4096], BF, tag="imA")
            imB = impool.tile([109, 4096], BF, tag="imB")
            for imt in (imA, imB):
                nc.vector.memset(imt[0:1, :], 1.0)

            def im_dma(t_, imt):
                hi = bass.AP(tensor=xpad, offset=t_ * XPT,
                             ap=[[4096, 36], [1, 4096]])
                lo = bass.AP(tensor=xpad, offset=t_ * XPT + 36 * 4096,
                             ap=[[4096, 36], [1, 4096]])
                nc.sync.dma_start(imt[1:37, :], hi)
                nc.sync.dma_start(imt[37:73, :], hi)
                nc.sync.dma_start(imt[73:109, :], lo)

            def colmat(name, j, w=128):
                return csb[name][:, j * w:(j + 1) * w]

            im_dma(0, imA)

            # persistent-ish per-step tiles come from rotating pools
            prev = None  # state carried from step t-1 for the node path

            for t in range(T + 1):
                if t < T:
                    im = imA if t % 2 == 0 else imB
                    o0 = o0A if t % 2 == 0 else o0B
                    o0r = o0[:].rearrange("p (h w) -> p h w", h=34)

                # ========== conv0(t): matmuls + drains ==========
                if t < T:
                    s0t = bpool.tile([128, 4096], BF, tag="s0t")
                    p1 = bpool.tile([128, 2048], BF, tag="p1")
                    p1r = p1[:].rearrange("p (h w) -> p h w", h=64)
                    for c in range(8):
                        sl = slice(c * 512, (c + 1) * 512)
                        ps = ps_conv.tile([128, 512], FP, tag="pc")
                        nc.tensor.matmul(ps[:], csb["w0bd"][:], im[:, sl],
                                         start=True, stop=True)
                        if t == 0:
                            # no membrane yet: u == ps; state = (sgn-1)*u
                            nc.scalar.activation(s0t[:, sl], ps[:], Act.Sign,
                                                 bias=actc[:, 0:1])
                            nc.vector.scalar_tensor_tensor(
                                v0[:, sl], s0t[:, sl], 1.0, ps[:],
                                Alu.subtract, Alu.mult)
                        else:
                            # u = -(1-c0)/2 * state + conv  (state = (sgn-1)u)
                            nc.vector.scalar_tensor_tensor(
                                u0t[:, sl], v0[:, sl], -0.5 * l0, ps[:],
                                Alu.mult, Alu.add)
                            nc.scalar.activation(s0t[:, sl], u0t[:, sl],
                                                 Act.Sign, bias=actc[:, 0:1])
                            # reset, compare-free: state = sgn*u - u (GPSIMD)
                            tr0 = bpool.tile([128, 512], FP, tag="tr0")
                            nc.gpsimd.tensor_tensor(
                                tr0[:], s0t[:, sl], u0t[:, sl], Alu.mult)
                            nc.gpsimd.tensor_tensor(
                                v0[:, sl], tr0[:], u0t[:, sl], Alu.subtract)
                        s0r = s0t[:, sl].rearrange("p (h w) -> p h w", h=8)
                        nc.gpsimd.tensor_tensor(
                            p1r[:, c * 8:(c + 1) * 8, :],
                            s0r[:, :, 0::2], s0r[:, :, 1::2], Alu.add)

                    if t + 1 < T:
                        im_dma(t + 1, imB if t % 2 == 0 else imA)

                    # pool-V + back to true-sum domain (+f0sum for free)
                    pv = bpool.tile([128, 1024], BF, tag="pv")
                    nc.gpsimd.tensor_tensor(
                        pv[:], p1r[:, 0::2, :], p1r[:, 1::2, :], Alu.add)
                    f0sum = tpool.tile([128, 1], FP, tag="f0sum")
                    pvr = pv[:].rearrange("p (h w) -> p h w", h=32)
                    nc.vector.tensor_scalar(
                        o0r[:, 1:33, 1:33], pvr, 4.0, None, Alu.add,
                        op1=Alu.add, accum_out=f0sum[:])

                    if DBG and t == 0:
                        nc.sync.dma_start(dbg["d_s0"][:], s0t[:])
                        nc.sync.dma_start(dbg["d_o0"][:], o0[:])
                        nc.sync.dma_start(dbg["d_f0sum"][:], f0sum[:])
                    if DBG and t == 1:
                        nc.sync.dma_start(dbg["d_u1"][:], u0t[:])
                    # f0 = relu(ft @ f0sum/4096 + ftb)
                    psf0 = ps_tiny.tile([128, 1], FP, tag="gt")
                    nc.tensor.matmul(psf0[:], colmat("ftmm", 0), f0sum[:],
                                     start=True, stop=True)
                    # f04 = 0.4*relu(...) with the 0.4 folded into ftmm/ftb
                    f04 = tpool.tile([128, 1], FP, tag="f04")
                    nc.vector.tensor_scalar(f04[:], psf0[:], csb["ftb0"][:],
                                            0.0, Alu.add, op1=Alu.max)

                # ========== node path for t-1 ==========
                if prev is not None:
                    po0r, psw, pf04, pt = prev
                    s1t = bpool.tile([128, 3072], BF, tag="s1t")
                    snsum = tpool.tile([128, 6], FP, tag="snsum")
                    for n in range(3):
                        for c in range(2):
                            psn = ps_node.tile([128, 512], FP, tag="pn")
                            for k in range(9):
                                dy, dx = k // 3, k % 3
                                rhs = po0r[:, dy + 16 * c: dy + 16 * c + 16,
                                           dx:dx + 32]
                                nc.tensor.matmul(
                                    psn[:], psw[n][:, k * 128:(k + 1) * 128],
                                    rhs, start=(k == 0),
                                    stop=(k == 8 and not biasn_nz))
                            if biasn_nz:
                                nc.tensor.matmul(
                                    psn[:],
                                    csb["biasn"][0:1, n * 128:(n + 1) * 128],
                                    ones[:], start=False, stop=True)
                            sl = slice(n * 1024 + c * 512,
                                       n * 1024 + (c + 1) * 512)
                            if pt == 0:
                                nc.scalar.activation(
                                    s1t[:, sl], psn[:], Act.Sign,
                                    bias=actc[:, 0:1],
                                    accum_out=snsum[:, n * 2 + c:
                                                    n * 2 + c + 1])
                                nc.vector.scalar_tensor_tensor(
                                    vn[:, sl], s1t[:, sl], 1.0, psn[:],
                                    Alu.subtract, Alu.mult)
                            else:
                                nc.vector.scalar_tensor_tensor(
                                    unt[:, sl], vn[:, sl], -0.5 * lns[n],
                                    psn[:], Alu.mult, Alu.add)
                                nc.scalar.activation(
                                    s1t[:, sl], unt[:, sl], Act.Sign,
                                    bias=actc[:, 0:1],
                                    accum_out=snsum[:, n * 2 + c:
                                                    n * 2 + c + 1])
                                trn = bpool.tile([128, 512], FP, tag="trn")
                                nc.gpsimd.tensor_tensor(
                                    trn[:], s1t[:, sl], unt[:, sl], Alu.mult)
                                nc.gpsimd.tensor_tensor(
                                    vn[:, sl], trn[:], unt[:, sl],
                                    Alu.subtract)

                    if DBG and pt == 0:
                        nc.sync.dma_start(dbg["d_s1"][:], s1t[:])
                        nc.sync.dma_start(dbg["d_snsum"][:], snsum[:])
                    # ---- y(t-1): exact fp32 accumulation in PSUM via
                    # scaled-identity matmuls; DMA straight from PSUM
                    for c in range(2):
                        psy = ps_node.tile([128, 512], FP, tag="pn")
                        nc.tensor.matmul(
                            psy[:], csb["iy"][:, 0:128],
                            po0r[:, 1 + 16 * c:17 + 16 * c, 1:33],
                            start=True, stop=False)
                        for n in range(3):
                            nc.tensor.matmul(
                                psy[:], csb["iy"][:, (n + 1) * 128:
                                                  (n + 2) * 128],
                                s1t[:, n * 1024 + c * 512:
                                    n * 1024 + (c + 1) * 512],
                                start=False, stop=False)
                        nc.tensor.matmul(psy[:], csb["ycrow"][:], yones[:],
                                         start=False, stop=True)
                        ysb = bpool.tile([128, 512], FP, tag="ysb")
                        nc.scalar.activation(ysb[:], psy[:], Act.Copy,
                                             bias=0.0)
                        nc.sync.dma_start(
                            bass.AP(tensor=y,
                                    offset=(t - 1) * BC * CO * 1024 + c * 512,
                                    ap=[[1024, 128], [1, 512]]),
                            ysb[:])

                    # ---- feats(t-1) + full trace update
                    if t >= T:
                        break
                    psf = ps_tiny.tile([128, 3], FP, tag="gt")
                    nc.tensor.matmul(psf[:], colmat("ftmm", 1),
                                     snsum[:, 0::2], start=True, stop=False)
                    nc.tensor.matmul(psf[:], colmat("ftmm", 1),
                                     snsum[:, 1::2], start=False, stop=True)
                    fn04 = tpool.tile([128, 3], FP, tag="fn04")
                    nc.vector.tensor_scalar(fn04[:], psf[:], csb["ftb1"][:],
                                            0.0, Alu.add, op1=Alu.max)
                    nc.vector.scalar_tensor_tensor(
                        Tt[:, 0:1], Tt[:, 0:1], DECAY, pf04[:],
                        Alu.mult, Alu.add)
                    nc.vector.scalar_tensor_tensor(
                        Tt[:, 1:4], Tt[:, 1:4], DECAY, fn04[:],
                        Alu.mult, Alu.add)

                if t >= T:
                    break

                # trace row-0 pre-update with f0(t)
                nc.vector.scalar_tensor_tensor(
                    Tt[:, 0:1], Tt[:, 0:1], DECAY, f04[:], Alu.mult, Alu.add)
                if DBG and t == 0:
                    nc.sync.dma_start(dbg["d_f0t"][:], f04[:])
                    nc.sync.dma_start(dbg["d_Tt"][:], Tt[:])

                # ========== graph math (t) ==========
                def tiny(tag, p_, f_):
                    return tpool.tile([p_, f_], FP, tag=tag, name=tag)

                pse12 = ps_tiny.tile([40, 4], FP, tag="gt")
                nc.tensor.matmul(pse12[:], csb["m12"][:], Tt[:],
                                 start=True, stop=True)

                def reap(ap_, tail):
                    dims = [list(d) for d in ap_.ap][:-1] + tail
                    return bass.AP(tensor=ap_.tensor, offset=ap_.offset,
                                   ap=dims)

                def bc_n(ap_):  # [p,4] -> free (n,m): n varies, m bcast
                    return reap(ap_, [[1, 4], [0, 4]])

                def bc_m(ap_):  # free (n,m): n bcast, m varies
                    return reap(ap_, [[0, 4], [1, 4]])

                e2t = tiny("e2t", 8, 4)
                nc.vector.tensor_copy(e2t[:], pse12[32:40, :])
                es = tiny("es", 8, 16)
                nc.vector.tensor_tensor(es[:], bc_n(pse12[0:8, :]),
                                        bc_m(e2t[:]), Alu.add)
                es2 = tiny("es2", 8, 16)
                nc.vector.tensor_scalar_mul(es2[:], es[:], 0.2)
                el = tiny("el", 8, 16)
                nc.vector.tensor_tensor(el[:], es[:], es2[:], Alu.max)

                if DBG and t == 0:
                    nc.sync.dma_start(dbg["d_es"][:], es[:])
                psE = ps_tiny.tile([2, 16], FP, tag="gt")
                nc.tensor.matmul(psE[:], csb["ghbd"][:], el[:],
                                 start=True, stop=True)
                Ec = tiny("Ec", 2, 16)
                nc.vector.tensor_copy(Ec[:], psE[:])

                def tr_nm(ap_):  # read transposed over (n,m)
                    return reap(ap_, [[1, 4], [4, 4]])

                L = tiny("L", 2, 16)
                nc.vector.tensor_tensor(L[:], Ec[:], tr_nm(Ec[:]), Alu.add)
                Lr = L[:].rearrange("p (n m) -> p n m", n=4)
                mx = tiny("mx", 2, 4)
                nc.vector.tensor_reduce(mx[:], Lr, mybir.AxisListType.X,
                                        Alu.max)
                xm = tiny("xm", 2, 16)
                nc.vector.tensor_tensor(xm[:], L[:], bc_n(mx[:]), Alu.subtract)
                ex = tiny("ex", 2, 16)
                nc.scalar.activation(ex[:], xm[:], Act.Exp,
                                     bias=actc[0:2, 2:3])
                sm = tiny("sm", 2, 4)
                exr = ex[:].rearrange("p (n m) -> p n m", n=4)
                nc.vector.tensor_reduce(sm[:], exr, mybir.AxisListType.X,
                                        Alu.add)
                rc = tiny("rc", 2, 4)
                nc.vector.reciprocal(rc[:], sm[:])
                S = tiny("S", 2, 16)
                nc.vector.tensor_tensor(S[:], ex[:], bc_n(rc[:]), Alu.mult)

                if DBG and t == 0:
                    nc.sync.dma_start(dbg["d_S"][:], S[:])
                Sr = S[:].rearrange("p (n m) -> p n m", n=4)
                lo = tiny("lo", 2, 8)
                lor = lo[:].rearrange("p (n m) -> p n m", n=4)
                hi = tiny("hi", 2, 8)
                hir = hi[:].rearrange("p (n m) -> p n m", n=4)
                nc.vector.tensor_tensor(lor, Sr[:, :, 0::2], Sr[:, :, 1::2],
                                        Alu.min)
                nc.vector.tensor_tensor(hir, Sr[:, :, 0::2], Sr[:, :, 1::2],
                                        Alu.max)
                kth = tiny("kth", 2, 4)
                l2 = tiny("l2", 2, 4)
                nc.vector.tensor_tensor(l2[:], lor[:, :, 0], lor[:, :, 1],
                                        Alu.max)
                h2 = tiny("h2", 2, 4)
                nc.vector.tensor_tensor(h2[:], hir[:, :, 0], hir[:, :, 1],
                                        Alu.min)
                nc.vector.tensor_tensor(kth[:], l2[:], h2[:], Alu.min)
                msk = tiny("msk", 2, 16)
                nc.vector.tensor_tensor(msk[:], S[:], bc_n(kth[:]), Alu.is_ge)
                Sp = tiny("Sp", 2, 16)
                nc.vector.tensor_tensor(Sp[:], S[:], msk[:], Alu.mult)

                A2 = tiny("A2", 2, 16)
                nc.vector.tensor_tensor(A2[:], Sp[:], tr_nm(Sp[:]), Alu.add)
                rs = tiny("rs", 2, 4)
                A2r = A2[:].rearrange("p (n m) -> p n m", n=4)
                nc.vector.tensor_reduce(rs[:], A2r, mybir.AxisListType.X,
                                        Alu.add)
                # q = rsqrt(z), z = 0.5*rs + 1e-6: reciprocal-seeded
                # Newton (y0 = (1/z + 1)/2, ~12% max err on z in [0.35,1.6];
                # two iterations of y *= 1.5 - 0.5*z*y^2 -> ~1e-3 max err)
                zq = tiny("zq", 2, 4)
                nc.vector.tensor_scalar(zq[:], rs[:], 0.5, 1e-6, Alu.mult,
                                        op1=Alu.add)
                rcq = tiny("rcq", 2, 4)
                nc.vector.reciprocal(rcq[:], zq[:])
                q = tiny("q", 2, 4)
                nc.vector.tensor_scalar(q[:], rcq[:], 0.5, 0.5, Alu.mult,
                                        op1=Alu.add)
                for _ in range(2):
                    t1q = tiny("t1q", 2, 4)
                    nc.vector.tensor_tensor(t1q[:], q[:], q[:], Alu.mult)
                    nc.vector.tensor_tensor(t1q[:], t1q[:], zq[:], Alu.mult)
                    nc.vector.tensor_scalar(t1q[:], t1q[:], -0.5, 1.5,
                                            Alu.mult, op1=Alu.add)
                    nc.vector.tensor_tensor(q[:], q[:], t1q[:], Alu.mult)

                t1 = tiny("t1", 2, 16)
                nc.vector.tensor_tensor(t1[:], A2[:], bc_n(q[:]), Alu.mult)
                OPt = tiny("OPt", 2, 16)
                nc.vector.scalar_tensor_tensor(OPt[:], t1[:], 0.5, bc_m(q[:]),
                                               Alu.mult, Alu.mult)
                col0 = reap(OPt[:], [[0, 4], [4, 4]])
                t2 = tiny("t2", 2, 16)
                nc.vector.tensor_tensor(t2[:], OPt[:], col0, Alu.mult)
                af = tiny("af", 2, 4)
                t2r = t2[:].rearrange("p (n m) -> p n m", n=4)
                nc.vector.tensor_reduce(af[:], t2r, mybir.AxisListType.X,
                                        Alu.add)
                al3 = tiny("al3", 2, 3)
                nc.vector.tensor_tensor(al3[:], af[:, 1:4], csb["cnrow"][:],
                                        Alu.mult)
                psb = ps_tiny.tile([128, 3], FP, tag="gt")
                nc.tensor.matmul(psb[:], csb["gbc"][:], al3[:],
                                 start=True, stop=True)
                aap = tiny("aap", 128, 3)
                nc.vector.tensor_copy(aap[:], psb[:])

                # alpha-scaled node weights; sw0 gates the next node phase,
                # so compute it on DVE straight from PSUM
                sw = [swpool.tile([128, 9 * 128], BF, tag=f"sw{n}",
                                  name=f"sw{n}") for n in range(3)]
                nc.vector.tensor_scalar_mul(
                    sw[0][:, 0:128], csb["wnod"][:, 0:128], aap[:, 0:1])
                nc.vector.tensor_scalar_mul(
                    sw[0][:, 128:9 * 128], csb["wnod"][:, 128:9 * 128],
                    aap[:, 0:1])
                if DBG and t == 0:
                    nc.sync.dma_start(dbg["d_aap"][:], aap[:])
                    nc.sync.dma_start(dbg["d_q"][:], q[:])
                for n in (1, 2):
                    nc.scalar.activation(
                        sw[n][:],
                        csb["wnod"][:, n * 9 * 128:(n + 1) * 9 * 128],
                        Act.Copy, scale=aap[:, n:n + 1])

                prev = (o0r, sw, f04, t)

    if not nc.is_finalized():
        nc.finalize()
    return nc


_NC_CACHE = {}


def _get_nc(yw, biasn_nz):
    key = (tuple(float(v) for v in yw), biasn_nz)
    if key not in _NC_CACHE:
        _NC_CACHE[key] = build_nc(yw, biasn_nz)
    return _NC_CACHE[key]


def _make_xpad(xcore):
    """[T,BC,CIN,64,64] f32 -> host im2col, flat bf16 [T*72*4096].

    Rows 0-35 of each step: bf16-hi of the zero-padded shifted x;
    rows 36-71: bf16 of the residual (x - hi). Row dy*12+dx*4+(b*2+ci)."""
    xp = np.zeros((T, 4, 66, 66), np.float32)
    xp[:, :, 1:65, 1:65] = xcore.reshape(T, 4, 64, 64)
    xim = np.empty((T, 72, 4096), ml_dtypes.bfloat16)
    for dy in range(3):
        for dx in range(3):
            blk = xp[:, :, dy:dy + 64, dx:dx + 64].reshape(T, 4, 4096)
            hi = blk.astype(ml_dtypes.bfloat16)
            lo = (blk - hi.astype(np.float32)).astype(ml_dtypes.bfloat16)
            r = dy * 12 + dx * 4
            xim[:, r:r + 4] = hi
            xim[:, 36 + r:36 + r + 4] = lo
    return np.ascontiguousarray(xim.reshape(-1))


def kernel(**inputs):
    x = np.asarray(inputs["x"], np.float32)
    consts, yw, (biasn_nz,) = _host_consts(
        inputs["conv0_w"], inputs["bn0_g"], inputs["bn0_b"], inputs["bn0_m"],
        inputs["bn0_v"], inputs["lif0_w"], inputs["convs_w"], inputs["bns_g"],
        inputs["bns_b"], inputs["bns_m"], inputs["bns_v"], inputs["lifs_w"],
        inputs["ft_w"], inputs["ft_b"], inputs["gat_w"], inputs["gat_a"],
        inputs["out_weights"])
    consts = {k: np.ascontiguousarray(v) for k, v in consts.items()}
    nc = _get_nc(yw, biasn_nz)
    core_ids = list(range(NCORES))
    in_maps = []
    for k in core_ids:
        m = dict(consts)
        m["xpad"] = _make_xpad(x[:, k * BC:(k + 1) * BC])
        in_maps.append(m)
    res = run_bass_kernel_spmd(nc, in_maps, core_ids).results
    out = np.concatenate([np.asarray(res[k]["y"]) for k in core_ids], axis=1)
    return out.astype(np.float32)


# revision 13
# speedup vs baseline: 1.0057x; 1.0057x over previous
"""STSPBlock Trainium2 kernel, v2.

Per core (batch-sharded B=16 -> 8 cores x B=2), partitions p = b*64+c.

Key points vs v1 (685us -> ~205us est., hw rel err 3.4e-3):
  - conv0 + node convs run in bf16 (1 cyc/row vs 4 for fp32).
  - conv0 gets near-fp32 precision for free by packing a split-bf16
    product into spare K rows: K=109 = ones + Whi*xhi + Wlo*xhi +
    Whi*xlo (im2col built host-side, one 8KB-run DMA per block).
    Node-conv bf16 weight rounding was measured to flip zero spikes
    (integer-valued rhs -> wide thresholds margins), so single-pass.
  - LIF membranes stay exact fp32: u = -(1-c)/2*state + conv via one
    DVE scalar_tensor_tensor reading PSUM (replaces identity matmuls,
    whose reduced-precision would flip spikes; fp32 matmul too slow).
    State is kept as (sign-1)*u so the GPSIMD reset needs only
    mult/subtract tensor_tensor ops (Pool HW has no compare/scalar).
  - spikes computed on ACT as sign(u-1) in {-1,+1}; consumers are
    linear so affine corrections fold into host constants. accum_out
    of the sign op yields the per-node spike counts for free.
  - y accumulates in PSUM fp32 via scaled-identity bf16 matmuls; the
    constant row is built from the *rounded* per-node scales (split
    hi+lo) so no-spike pixels cancel exactly.
  - alpha = f(graph attention) needs <1e-3 accuracy: rsqrt computed
    by reciprocal-seeded Newton (2 iters, mult-only; DVE pow and the
    bit-trick are not HW-legal, Ln/Exp thrash activation tables).
  - graph chain shortened (gat_w @ gat_a folded host-side), single
    activation table load, bias matmuls skipped when zero.
"""

import numpy as np
import ml_dtypes

import concourse.bass as bass
import concourse.bacc as bacc
import concourse.mybir as mybir
from concourse.tile import TileContext
from concourse.bass_utils import run_bass_kernel_spmd

FP = mybir.dt.float32
FR = mybir.dt.float32r
BF = mybir.dt.bfloat16
Alu = mybir.AluOpType
Act = mybir.ActivationFunctionType

T, BFULL, CIN, H, W = 8, 16, 2, 64, 64
CO, NN, HEADS = 64, 4, 4
HP, WP = 32, 32
BC = 2
NCORES = 8
EPS = 1e-5
DECAY = 0.6
HD = CO // HEADS
XPT = 72 * 4096         # host-im2col rows (hi 36 + lo 36) per step


# ----------------------------------------------------------------- host consts
def _host_consts(conv0_w, bn0_g, bn0_b, bn0_m, bn0_v, lif0_w,
                 convs_w, bns_g, bns_b, bns_m, bns_v, lifs_w,
                 ft_w, ft_b, gat_w, gat_a, out_weights):
    f32 = np.float32
    bf = ml_dtypes.bfloat16
    sig = lambda z: 1.0 / (1.0 + np.exp(-np.asarray(z, np.float64)))
    c0 = f32(sig(lif0_w))
    cn = sig(lifs_w).astype(f32)          # [3]
    ws = sig(out_weights).astype(f32)     # [4]

    s0c = (bn0_g / np.sqrt(bn0_v + EPS)).astype(f32)
    bias0 = ((bn0_b - bn0_m * s0c) * c0).astype(f32)
    W0f = (conv0_w * s0c[:, None, None, None] * c0).astype(f32)  # [64,2,3,3]

    # conv0 folded lhsT [109, 128], split-bf16 via spare K rows:
    #   rows 1-36:  Whi (pairs with xhi), rows 37-72: Wlo (pairs with xhi),
    #   rows 73-108: Whi (pairs with xlo); row 0 = bias (ones row).
    # Dropped Wlo*xlo term is O(2^-16) relative.
    W0hi = W0f.astype(bf).astype(f32)
    W0lo = (W0f - W0hi).astype(bf).astype(f32)
    w0bd = np.zeros((109, 128), f32)
    for dy in range(3):
        for dx in range(3):
            for b in range(2):
                for ci in range(2):
                    p = 1 + dy * 12 + dx * 4 + b * 2 + ci
                    w0bd[p, b * 64:(b + 1) * 64] = W0hi[:, ci, dy, dx]
                    w0bd[36 + p, b * 64:(b + 1) * 64] = W0lo[:, ci, dy, dx]
                    w0bd[72 + p, b * 64:(b + 1) * 64] = W0hi[:, ci, dy, dx]
    w0bd[0, 0:64] = bias0
    w0bd[0, 64:128] = bias0

    sncol = (bns_g / np.sqrt(bns_v + EPS)).astype(f32)            # [3,64]
    biasn_raw = (bns_b - bns_m * sncol).astype(f32)               # [3,64]
    # 0.125 = avgpool fold (out0p holds 2x the SUM of 4 spikes)
    Wf = (convs_w * sncol[:, :, None, None, None] * 0.125).astype(f32)

    wnod = np.zeros((3, 9, 128, 128), f32)
    for n in range(3):
        for dy in range(3):
            for dx in range(3):
                k = dy * 3 + dx
                blk = Wf[n, :, :, dy, dx].T    # [ci, co]
                wnod[n, k, 0:64, 0:64] = blk
                wnod[n, k, 64:128, 64:128] = blk

    biasn = np.concatenate([np.tile(cn[n] * biasn_raw[n], 2)
                            for n in range(3)]).reshape(1, 384).astype(f32)

    def bd(m):  # block-diag [128,128] of m.T twice ([co,ci] -> lhsT)
        z = np.zeros((128, 128), f32)
        z[0:64, 0:64] = m.T
        z[64:128, 64:128] = m.T
        return z

    # f0: pre-relu = ft_w @ (f0sum/4096) + ft_b          (f0sum: true sums)
    # fn: pre-relu = ft_w @ (snsum/2048) + ft_b + 0.5*ft_w.sum(1)
    #     (snsum = sum over pixels of sign values in {-1,+1})
    ftmm = np.stack([bd(ft_w * (0.4 / 8192.0)), bd(ft_w * (0.4 / 2048.0))])
    ftb0 = (0.4 * np.tile(ft_b, 2)).reshape(128, 1).astype(f32)
    ftb1 = (0.4 * np.tile(ft_b + 0.5 * ft_w.sum(axis=1), 2)).reshape(
        128, 1).astype(f32)

    gwbd = bd(gat_w).astype(f32)
    ga1 = np.zeros((128, 8), f32)
    ga2 = np.zeros((128, 8), f32)
    for b in range(2):
        for h in range(HEADS):
            for d in range(HD):
                ga1[b * 64 + h * 16 + d, b * 4 + h] = gat_a[h, d]
                ga2[b * 64 + h * 16 + d, b * 4 + h] = gat_a[h, HD + d]
    M1 = (gwbd @ ga1).astype(f32)          # [128, 8] folded gat_w @ a_l
    M2 = (gwbd @ ga2).astype(f32)
    # one MM: out partitions 0-7 = e1, 32-39 = e2 (PSUM reads need
    # 32-aligned base partitions)
    M12 = np.zeros((128, 40), f32)
    M12[:, 0:8] = M1
    M12[:, 32:40] = M2

    ghbd = np.zeros((8, 2), f32)
    for b in range(2):
        ghbd[b * 4:(b + 1) * 4, b] = 12.5

    gbc = np.zeros((2, 128), f32)
    gbc[0, 0:64] = 1.0
    gbc[1, 64:128] = 1.0

    cnrow = np.tile(cn[None, :], (2, 1)).astype(f32)              # [2,3]

    def cols(stk):  # [k,128,128] -> [128, k*128]
        return np.ascontiguousarray(
            np.transpose(stk, (1, 0, 2)).reshape(128, -1))

    # y identity scales are bf16 in the MM; the constant row must cancel
    # the ROUNDED per-node scales exactly for no-spike pixels, so build it
    # from the bf16-rounded values and store it split hi+lo (K=2 rows).
    wnb = [f32(bf(f32(ws[n] * 0.5))) for n in (1, 2, 3)]
    iy = np.stack([float(ws[0]) * 0.125 * np.eye(128),
                   wnb[0] * np.eye(128),
                   wnb[1] * np.eye(128),
                   wnb[2] * np.eye(128)]).astype(f32)
    yC = f32(wnb[0]) + f32(wnb[1]) + f32(wnb[2])
    yChi = f32(bf(yC))
    yClo = f32(bf(yC - yChi))
    ycrow = np.zeros((2, 128), f32)
    ycrow[0, :] = yChi
    ycrow[1, :] = yClo

    consts = dict(
        w0bd=w0bd.astype(bf),
        wnod=cols(wnod.reshape(27, 128, 128)).astype(bf),
        biasn=biasn,
        ftmm=cols(ftmm), ftb0=ftb0, ftb1=ftb1,
        m12=M12, ghbd=ghbd, gbc=gbc, cnrow=cnrow,
        iy=cols(iy).astype(bf), ycrow=ycrow.astype(bf))

    # y scales: y = ws0*(out0sum/4) + sum_n wsn*(sn_sign+1)/2
    # plus LIF leak factors for the on-chip u-ops
    yw = (float(ws[0]) * 0.125,
          float(ws[1]) * 0.5, float(ws[2]) * 0.5, float(ws[3]) * 0.5,
          float(0.5 * (ws[1] + ws[2] + ws[3])),
          float(1.0 - c0), float(1.0 - cn[0]), float(1.0 - cn[1]),
          float(1.0 - cn[2]))
    flags = (bool(np.any(np.abs(biasn) > 0)),)
    return consts, yw, flags


CONST_SPECS = dict(w0bd=((109, 128), BF),
                   wnod=((128, 27 * 128), BF),
                   biasn=((1, 384), FP), ftmm=((128, 2 * 128), FP),
                   ftb0=((128, 1), FP), ftb1=((128, 1), FP),
                   m12=((128, 40), FP),
                   ghbd=((8, 2), FP), gbc=((2, 128), FP),
                   cnrow=((2, 3), FP), iy=((128, 4 * 128), BF),
                   ycrow=((2, 128), BF))


# ------------------------------------------------------------------ the module
DBG = False


def build_nc(yw, biasn_nz):
    nc = bacc.Bacc(None, target_bir_lowering=False)
    xpad = nc.declare_dram_parameter("xpad", [T * XPT], BF, isOutput=False)
    cst = {k: nc.declare_dram_parameter(k, list(shp), dt, isOutput=False)
           for k, (shp, dt) in CONST_SPECS.items()}
    y = nc.declare_dram_parameter("y", [T, BC, CO, HP, WP], FP, isOutput=True)
    dbg = {}
    if DBG:
        for nm, shp, dt in [("d_s0", [128, 4096], BF), ("d_o0", [128, 1156], BF),
                        ("d_f0sum", [128, 1], FP), ("d_f0t", [128, 1], FP),
                        ("d_Tt", [128, 4], FP), ("d_es", [8, 16], FP),
                        ("d_S", [2, 16], FP), ("d_aap", [128, 3], FP),
                        ("d_q", [2, 4], FP), ("d_s1", [128, 3072], BF),
                        ("d_snsum", [128, 6], FP), ("d_u1", [128, 4096], FP)]:
            dbg[nm] = nc.declare_dram_parameter(nm, shp, dt, isOutput=True)

    w0s4, w1, w2, w3, yC, l0, ln1, ln2, ln3 = yw
    wns = (w1, w2, w3)
    lns = (ln1, ln2, ln3)

    with TileContext(nc) as tc:
        with (
            tc.tile_pool(name="consts", bufs=1) as cpool,
            tc.tile_pool(name="state", bufs=1) as spool,
            tc.tile_pool(name="im", bufs=2) as impool,
            tc.tile_pool(name="big", bufs=2) as bpool,
            tc.tile_pool(name="sw", bufs=2) as swpool,
            tc.tile_pool(name="tiny", bufs=4) as tpool,
            tc.tile_pool(name="pconv", bufs=5, space="PSUM") as ps_conv,
            tc.tile_pool(name="pnode", bufs=2, space="PSUM") as ps_node,
            tc.tile_pool(name="ptiny", bufs=1, space="PSUM") as ps_tiny,
        ):
            # ---- consts to SBUF (w0bd first on sync; rest on ACT queue
            # so the first conv0 + im DMAs aren't stuck behind them)
            csb = {}
            for k, (shp, dt) in CONST_SPECS.items():
                t_ = cpool.tile(list(shp), dt, tag=k)
                (nc.sync if k == "w0bd" else nc.scalar).dma_start(
                    t_[:], cst[k][:])
                csb[k] = t_

            ones = None
            if biasn_nz:
                ones = cpool.tile([1, 512], FP, tag="ones")
                nc.vector.memset(ones[:], 1.0)

            # activation biases must be APs: [-1.0 (sign), 1e-6 (ln), 0.0]
            actc = cpool.tile([128, 3], FP, tag="actc")
            nc.vector.memset(actc[:, 0:1], -1.0)
            nc.vector.memset(actc[:, 1:2], 1e-6)
            nc.vector.memset(actc[:, 2:3], 0.0)

            # ---- states (in-place: u-op reads v before reset rewrites it)
            v0 = spool.tile([128, 4096], FP, tag="v0")
            vn = spool.tile([128, 3072], FP, tag="vn")
            u0t = spool.tile([128, 4096], FP, tag="u0t")
            unt = spool.tile([128, 3072], FP, tag="unt")
            Tt = spool.tile([128, 4], FP, tag="Tt")
            nc.vector.memset(Tt[:], 0.0)
            nc.vector.memset(vn[:], 0.0)

            # out0p (true spike-sum domain, zero border = zero pad)
            o0A = spool.tile([128, 34 * 34], BF, tag="o0A")
            o0B = spool.tile([128, 34 * 34], BF, tag="o0B")
            nc.vector.memset(o0A[:], 0.0)
            nc.vector.memset(o0B[:], 0.0)
            yones = cpool.tile([2, 512], BF, tag="yones")
            nc.vector.memset(yones[:], 1.0)

            imA = impool.tile([109, 4096], BF, tag="imA")
            imB = impool.tile([109, 4096], BF, tag="imB")
            for imt in (imA, imB):
                nc.vector.memset(imt[0:1, :], 1.0)

            def im_dma(t_, imt):
                hi = bass.AP(tensor=xpad, offset=t_ * XPT,
                             ap=[[4096, 36], [1, 4096]])
                lo = bass.AP(tensor=xpad, offset=t_ * XPT + 36 * 4096,
                             ap=[[4096, 36], [1, 4096]])
                nc.sync.dma_start(imt[1:37, :], hi)
                nc.sync.dma_start(imt[37:73, :], hi)
                nc.sync.dma_start(imt[73:109, :], lo)

            def colmat(name, j, w=128):
                return csb[name][:, j * w:(j + 1) * w]

            im_dma(0, imA)

            # persistent-ish per-step tiles come from rotating pools
            prev = None  # state carried from step t-1 for the node path

            for t in range(T + 1):
                if t < T:
                    im = imA if t % 2 == 0 else imB
                    o0 = o0A if t % 2 == 0 else o0B
                    o0r = o0[:].rearrange("p (h w) -> p h w", h=34)

                # ========== conv0(t): matmuls + drains ==========
                if t < T:
                    s0t = bpool.tile([128, 4096], BF, tag="s0t")
                    p1 = bpool.tile([128, 2048], BF, tag="p1")
                    p1r = p1[:].rearrange("p (h w) -> p h w", h=64)
                    for c in range(8):
                        sl = slice(c * 512, (c + 1) * 512)
                        ps = ps_conv.tile([128, 512], FP, tag="pc")
                        nc.tensor.matmul(ps[:], csb["w0bd"][:], im[:, sl],
                                         start=True, stop=True)
                        if t == 0:
                            # no membrane yet: u == ps; state = (sgn-1)*u
                            nc.scalar.activation(s0t[:, sl], ps[:], Act.Sign,
                                                 bias=actc[:, 0:1])
                            nc.vector.scalar_tensor_tensor(
                                v0[:, sl], s0t[:, sl], 1.0, ps[:],
                                Alu.subtract, Alu.mult)
                        else:
                            # u = -(1-c0)/2 * state + conv  (state = (sgn-1)u)
                            nc.vector.scalar_tensor_tensor(
                                u0t[:, sl], v0[:, sl], -0.5 * l0, ps[:],
                                Alu.mult, Alu.add)
                            nc.scalar.activation(s0t[:, sl], u0t[:, sl],
                                                 Act.Sign, bias=actc[:, 0:1])
                            # reset, compare-free: state = sgn*u - u (GPSIMD)
                            tr0 = bpool.tile([128, 512], FP, tag="tr0")
                            nc.gpsimd.tensor_tensor(
                                tr0[:], s0t[:, sl], u0t[:, sl], Alu.mult)
                            nc.gpsimd.tensor_tensor(
                                v0[:, sl], tr0[:], u0t[:, sl], Alu.subtract)
                        s0r = s0t[:, sl].rearrange("p (h w) -> p h w", h=8)
                        nc.gpsimd.tensor_tensor(
                            p1r[:, c * 8:(c + 1) * 8, :],
                            s0r[:, :, 0::2], s0r[:, :, 1::2], Alu.add)

                    if t + 1 < T:
                        im_dma(t + 1, imB if t % 2 == 0 else imA)

                    # pool-V + back to true-sum domain (+f0sum for free)
                    pv = bpool.tile([128, 1024], BF, tag="pv")
                    nc.gpsimd.tensor_tensor(
                        pv[:], p1r[:, 0::2, :], p1r[:, 1::2, :], Alu.add)
                    f0sum = tpool.tile([128, 1], FP, tag="f0sum")
                    pvr = pv[:].rearrange("p (h w) -> p h w", h=32)
                    nc.vector.tensor_scalar(
                        o0r[:, 1:33, 1:33], pvr, 4.0, None, Alu.add,
                        op1=Alu.add, accum_out=f0sum[:])

                    if DBG and t == 0:
                        nc.sync.dma_start(dbg["d_s0"][:], s0t[:])
                        nc.sync.dma_start(dbg["d_o0"][:], o0[:])
                        nc.sync.dma_start(dbg["d_f0sum"][:], f0sum[:])
                    if DBG and t == 1:
                        nc.sync.dma_start(dbg["d_u1"][:], u0t[:])
                    # f0 = relu(ft @ f0sum/4096 + ftb)
                    psf0 = ps_tiny.tile([128, 1], FP, tag="gt")
                    nc.tensor.matmul(psf0[:], colmat("ftmm", 0), f0sum[:],
                                     start=True, stop=True)
                    # f04 = 0.4*relu(...) with the 0.4 folded into ftmm/ftb
                    f04 = tpool.tile([128, 1], FP, tag="f04")
                    nc.vector.tensor_scalar(f04[:], psf0[:], csb["ftb0"][:],
                                            0.0, Alu.add, op1=Alu.max)

                # ========== node path for t-1 ==========
                if prev is not None:
                    po0r, psw, pf04, pt = prev
                    s1t = bpool.tile([128, 3072], BF, tag="s1t")
                    snsum = tpool.tile([128, 6], FP, tag="snsum")
                    for n in range(3):
                        for c in range(2):
                            psn = ps_node.tile([128, 512], FP, tag="pn")
                            for k in range(9):
                                dy, dx = k // 3, k % 3
                                rhs = po0r[:, dy + 16 * c: dy + 16 * c + 16,
                                           dx:dx + 32]
                                nc.tensor.matmul(
                                    psn[:], psw[n][:, k * 128:(k + 1) * 128],
                                    rhs, start=(k == 0),
                                    stop=(k == 8 and not biasn_nz))
                            if biasn_nz:
                                nc.tensor.matmul(
                                    psn[:],
                                    csb["biasn"][0:1, n * 128:(n + 1) * 128],
                                    ones[:], start=False, stop=True)
                            sl = slice(n * 1024 + c * 512,
                                       n * 1024 + (c + 1) * 512)
                            if pt == 0:
                                nc.scalar.activation(
                                    s1t[:, sl], psn[:], Act.Sign,
                                    bias=actc[:, 0:1],
                                    accum_out=snsum[:, n * 2 + c:
                                                    n * 2 + c + 1])
                                nc.vector.scalar_tensor_tensor(
                                    vn[:, sl], s1t[:, sl], 1.0, psn[:],
                                    Alu.subtract, Alu.mult)
                            else:
                                nc.vector.scalar_tensor_tensor(
                                    unt[:, sl], vn[:, sl], -0.5 * lns[n],
                                    psn[:], Alu.mult, Alu.add)
                                nc.scalar.activation(
                                    s1t[:, sl], unt[:, sl], Act.Sign,
                                    bias=actc[:, 0:1],
                                    accum_out=snsum[:, n * 2 + c:
                                                    n * 2 + c + 1])
                                trn = bpool.tile([128, 512], FP, tag="trn")
                                nc.gpsimd.tensor_tensor(
                                    trn[:], s1t[:, sl], unt[:, sl], Alu.mult)
                                nc.gpsimd.tensor_tensor(
                                    vn[:, sl], trn[:], unt[:, sl],
                                    Alu.subtract)

                    if DBG and pt == 0:
                        nc.sync.dma_start(dbg["d_s1"][:], s1t[:])
                        nc.sync.dma_start(dbg["d_snsum"][:], snsum[:])
                    # ---- y(t-1): exact fp32 accumulation in PSUM via
                    # scaled-identity matmuls; DMA straight from PSUM
                    for c in range(2):
                        psy = ps_node.tile([128, 512], FP, tag="pn")
                        nc.tensor.matmul(
                            psy[:], csb["iy"][:, 0:128],
                            po0r[:, 1 + 16 * c:17 + 16 * c, 1:33],
                            start=True, stop=False)
                        for n in range(3):
                            nc.tensor.matmul(
                                psy[:], csb["iy"][:, (n + 1) * 128:
                                                  (n + 2) * 128],
                                s1t[:, n * 1024 + c * 512:
                                    n * 1024 + (c + 1) * 512],
                                start=False, stop=False)
                        nc.tensor.matmul(psy[:], csb["ycrow"][:], yones[:],
                                         start=False, stop=True)
                        ysb = bpool.tile([128, 512], FP, tag="ysb")
                        nc.scalar.activation(ysb[:], psy[:], Act.Copy,
                                             bias=0.0)
                        nc.sync.dma_start(
                            bass.AP(tensor=y,
                                    offset=(t - 1) * BC * CO * 1024 + c * 512,
                                    ap=[[1024, 128], [1, 512]]),
                            ysb[:])

                    # ---- feats(t-1) + full trace update
                    if t >= T:
                        break
                    psf = ps_tiny.tile([128, 3], FP, tag="gt")
                    nc.tensor.matmul(psf[:], colmat("ftmm", 1),
                                     snsum[:, 0::2], start=True, stop=False)
                    nc.tensor.matmul(psf[:], colmat("ftmm", 1),
                                     snsum[:, 1::2], start=False, stop=True)
                    fn04 = tpool.tile([128, 3], FP, tag="fn04")
                    nc.vector.tensor_scalar(fn04[:], psf[:], csb["ftb1"][:],
                                            0.0, Alu.add, op1=Alu.max)
                    nc.vector.scalar_tensor_tensor(
                        Tt[:, 0:1], Tt[:, 0:1], DECAY, pf04[:],
                        Alu.mult, Alu.add)
                    nc.vector.scalar_tensor_tensor(
                        Tt[:, 1:4], Tt[:, 1:4], DECAY, fn04[:],
                        Alu.mult, Alu.add)

                if t >= T:
                    break

                # trace row-0 pre-update with f0(t)
                nc.vector.scalar_tensor_tensor(
                    Tt[:, 0:1], Tt[:, 0:1], DECAY, f04[:], Alu.mult, Alu.add)
                if DBG and t == 0:
                    nc.sync.dma_start(dbg["d_f0t"][:], f04[:])
                    nc.sync.dma_start(dbg["d_Tt"][:], Tt[:])

                # ========== graph math (t) ==========
                def tiny(tag, p_, f_):
                    return tpool.tile([p_, f_], FP, tag=tag, name=tag)

                pse12 = ps_tiny.tile([40, 4], FP, tag="gt")
                nc.tensor.matmul(pse12[:], csb["m12"][:], Tt[:],
                                 start=True, stop=True)

                def reap(ap_, tail):
                    dims = [list(d) for d in ap_.ap][:-1] + tail
                    return bass.AP(tensor=ap_.tensor, offset=ap_.offset,
                                   ap=dims)

                def bc_n(ap_):  # [p,4] -> free (n,m): n varies, m bcast
                    return reap(ap_, [[1, 4], [0, 4]])

                def bc_m(ap_):  # free (n,m): n bcast, m varies
                    return reap(ap_, [[0, 4], [1, 4]])

                e2t = tiny("e2t", 8, 4)
                nc.vector.tensor_copy(e2t[:], pse12[32:40, :])
                es = tiny("es", 8, 16)
                nc.vector.tensor_tensor(es[:], bc_n(pse12[0:8, :]),
                                        bc_m(e2t[:]), Alu.add)
                es2 = tiny("es2", 8, 16)
                nc.vector.tensor_scalar_mul(es2[:], es[:], 0.2)
                el = tiny("el", 8, 16)
                nc.vector.tensor_tensor(el[:], es[:], es2[:], Alu.max)

                if DBG and t == 0:
                    nc.sync.dma_start(dbg["d_es"][:], es[:])
                psE = ps_tiny.tile([2, 16], FP, tag="gt")
                nc.tensor.matmul(psE[:], csb["ghbd"][:], el[:],
                                 start=True, stop=True)
                Ec = tiny("Ec", 2, 16)
                nc.vector.tensor_copy(Ec[:], psE[:])

                def tr_nm(ap_):  # read transposed over (n,m)
                    return reap(ap_, [[1, 4], [4, 4]])

                L = tiny("L", 2, 16)
                nc.vector.tensor_tensor(L[:], Ec[:], tr_nm(Ec[:]), Alu.add)
                Lr = L[:].rearrange("p (n m) -> p n m", n=4)
                mx = tiny("mx", 2, 4)
                nc.vector.tensor_reduce(mx[:], Lr, mybir.AxisListType.X,
                                        Alu.max)
                xm = tiny("xm", 2, 16)
                nc.vector.tensor_tensor(xm[:], L[:], bc_n(mx[:]), Alu.subtract)
                ex = tiny("ex", 2, 16)
                nc.scalar.activation(ex[:], xm[:], Act.Exp,
                                     bias=actc[0:2, 2:3])
                sm = tiny("sm", 2, 4)
                exr = ex[:].rearrange("p (n m) -> p n m", n=4)
                nc.vector.tensor_reduce(sm[:], exr, mybir.AxisListType.X,
                                        Alu.add)
                rc = tiny("rc", 2, 4)
                nc.vector.reciprocal(rc[:], sm[:])
                S = tiny("S", 2, 16)
                nc.vector.tensor_tensor(S[:], ex[:], bc_n(rc[:]), Alu.mult)

                if DBG and t == 0:
                    nc.sync.dma_start(dbg["d_S"][:], S[:])
                Sr = S[:].rearrange("p (n m) -> p n m", n=4)
                lo = tiny("lo", 2, 8)
                lor = lo[:].rearrange("p (n m) -> p n m", n=4)
                hi = tiny("hi", 2, 8)
                hir = hi[:].rearrange("p (n m) -> p n m", n=4)
                nc.vector.tensor_tensor(lor, Sr[:, :, 0::2], Sr[:, :, 1::2],
                                        Alu.min)
                nc.vector.tensor_tensor(hir, Sr[:, :, 0::2], Sr[:, :, 1::2],
                                        Alu.max)
                kth = tiny("kth", 2, 4)
                l2 = tiny("l2", 2, 4)
                nc.vector.tensor_tensor(l2[:], lor[:, :, 0], lor[:, :, 1],
                                        Alu.max)
                h2 = tiny("h2", 2, 4)
                nc.vector.tensor_tensor(h2[:], hir[:, :, 0], hir[:, :, 1],
                                        Alu.min)
                nc.vector.tensor_tensor(kth[:], l2[:], h2[:], Alu.min)
                msk = tiny("msk", 2, 16)
                nc.vector.tensor_tensor(msk[:], S[:], bc_n(kth[:]), Alu.is_ge)
                Sp = tiny("Sp", 2, 16)
                nc.vector.tensor_tensor(Sp[:], S[:], msk[:], Alu.mult)

                A2 = tiny("A2", 2, 16)
                nc.vector.tensor_tensor(A2[:], Sp[:], tr_nm(Sp[:]), Alu.add)
                rs = tiny("rs", 2, 4)
                A2r = A2[:].rearrange("p (n m) -> p n m", n=4)
                nc.vector.tensor_reduce(rs[:], A2r, mybir.AxisListType.X,
                                        Alu.add)
                # q = rsqrt(z), z = 0.5*rs + 1e-6: reciprocal-seeded
                # Newton (y0 = (1/z + 1)/2, ~12% max err on z in [0.35,1.6];
                # two iterations of y *= 1.5 - 0.5*z*y^2 -> ~1e-3 max err)
                zq = tiny("zq", 2, 4)
                nc.vector.tensor_scalar(zq[:], rs[:], 0.5, 1e-6, Alu.mult,
                                        op1=Alu.add)
                rcq = tiny("rcq", 2, 4)
                nc.vector.reciprocal(rcq[:], zq[:])
                q = tiny("q", 2, 4)
                nc.vector.tensor_scalar(q[:], rcq[:], 0.408, 0.545, Alu.mult,
                                        op1=Alu.add)
                for _ in range(1):
                    t1q = tiny("t1q", 2, 4)
                    nc.vector.tensor_tensor(t1q[:], q[:], q[:], Alu.mult)
                    nc.vector.tensor_tensor(t1q[:], t1q[:], zq[:], Alu.mult)
                    nc.vector.tensor_scalar(t1q[:], t1q[:], -0.5, 1.5,
                                            Alu.mult, op1=Alu.add)
                    nc.vector.tensor_tensor(q[:], q[:], t1q[:], Alu.mult)

                t1 = tiny("t1", 2, 16)
                nc.vector.tensor_tensor(t1[:], A2[:], bc_n(q[:]), Alu.mult)
                OPt = tiny("OPt", 2, 16)
                nc.vector.scalar_tensor_tensor(OPt[:], t1[:], 0.5, bc_m(q[:]),
                                               Alu.mult, Alu.mult)
                col0 = reap(OPt[:], [[0, 4], [4, 4]])
                t2 = tiny("t2", 2, 16)
                nc.vector.tensor_tensor(t2[:], OPt[:], col0, Alu.mult)
                af = tiny("af", 2, 4)
                t2r = t2[:].rearrange("p (n m) -> p n m", n=4)
                nc.vector.tensor_reduce(af[:], t2r, mybir.AxisListType.X,
                                        Alu.add)
                al3 = tiny("al3", 2, 3)
                nc.vector.tensor_tensor(al3[:], af[:, 1:4], csb["cnrow"][:],
                                        Alu.mult)
                psb = ps_tiny.tile([128, 3], FP, tag="gt")
                nc.tensor.matmul(psb[:], csb["gbc"][:], al3[:],
                                 start=True, stop=True)
                aap = tiny("aap", 128, 3)
                nc.vector.tensor_copy(aap[:], psb[:])

                # alpha-scaled node weights; sw0 gates the next node phase,
                # so compute it on DVE straight from PSUM
                sw = [swpool.tile([128, 9 * 128], BF, tag=f"sw{n}",
                                  name=f"sw{n}") for n in range(3)]
                nc.vector.tensor_scalar_mul(
                    sw[0][:, 0:128], csb["wnod"][:, 0:128], aap[:, 0:1])
                nc.vector.tensor_scalar_mul(
                    sw[0][:, 128:9 * 128], csb["wnod"][:, 128:9 * 128],
                    aap[:, 0:1])
                if DBG and t == 0:
                    nc.sync.dma_start(dbg["d_aap"][:], aap[:])
                    nc.sync.dma_start(dbg["d_q"][:], q[:])
                for n in (1, 2):
                    nc.scalar.activation(
                        sw[n][:],
                        csb["wnod"][:, n * 9 * 128:(n + 1) * 9 * 128],
                        Act.Copy, scale=aap[:, n:n + 1])

                prev = (o0r, sw, f04, t)

    if not nc.is_finalized():
        nc.finalize()
    return nc


_NC_CACHE = {}


def _get_nc(yw, biasn_nz):
    key = (tuple(float(v) for v in yw), biasn_nz)
    if key not in _NC_CACHE:
        _NC_CACHE[key] = build_nc(yw, biasn_nz)
    return _NC_CACHE[key]


def _make_xpad(xcore):
    """[T,BC,CIN,64,64] f32 -> host im2col, flat bf16 [T*72*4096].

    Rows 0-35 of each step: bf16-hi of the zero-padded shifted x;
    rows 36-71: bf16 of the residual (x - hi). Row dy*12+dx*4+(b*2+ci)."""
    xp = np.zeros((T, 4, 66, 66), np.float32)
    xp[:, :, 1:65, 1:65] = xcore.reshape(T, 4, 64, 64)
    xim = np.empty((T, 72, 4096), ml_dtypes.bfloat16)
    for dy in range(3):
        for dx in range(3):
            blk = xp[:, :, dy:dy + 64, dx:dx + 64].reshape(T, 4, 4096)
            hi = blk.astype(ml_dtypes.bfloat16)
            lo = (blk - hi.astype(np.float32)).astype(ml_dtypes.bfloat16)
            r = dy * 12 + dx * 4
            xim[:, r:r + 4] = hi
            xim[:, 36 + r:36 + r + 4] = lo
    return np.ascontiguousarray(xim.reshape(-1))


def kernel(**inputs):
    x = np.asarray(inputs["x"], np.float32)
    consts, yw, (biasn_nz,) = _host_consts(
        inputs["conv0_w"], inputs["bn0_g"], inputs["bn0_b"], inputs["bn0_m"],
        inputs["bn0_v"], inputs["lif0_w"], inputs["convs_w"], inputs["bns_g"],
        inputs["bns_b"], inputs["bns_m"], inputs["bns_v"], inputs["lifs_w"],
        inputs["ft_w"], inputs["ft_b"], inputs["gat_w"], inputs["gat_a"],
        inputs["out_weights"])
    consts = {k: np.ascontiguousarray(v) for k, v in consts.items()}
    nc = _get_nc(yw, biasn_nz)
    core_ids = list(range(NCORES))
    in_maps = []
    for k in core_ids:
        m = dict(consts)
        m["xpad"] = _make_xpad(x[:, k * BC:(k + 1) * BC])
        in_maps.append(m)
    res = run_bass_kernel_spmd(nc, in_maps, core_ids).results
    out = np.concatenate([np.asarray(res[k]["y"]) for k in core_ids], axis=1)
    return out.astype(np.float32)
